# revision 8
# baseline (speedup 1.0000x reference)
"""Trainium2 Bass kernel for the neural-CA step (dense_cnn nn_CA_64948495450630).

Problem (hardcoded shapes): x, noise [4, 16, 512, 512] f32; w1 [128, 48];
b1 [128]; w2 [16, 128]; b2 [16].

    gx, gy, ci = depthwise 3x3 sobel_x / sobel_y / identity convs of x
    perc = concat([gx, gy, ci])                      # [B, 48, H, W]
    h1   = relu(w1 @ perc + b1)                      # per-pixel MLP
    upd  = (w2 @ h1 + b2) * (noise > 0.5)
    alive = (maxpool3x3(x[0, 3]) > 0.1)
    state = (x + upd) * alive ; visible = state[:, :4]

Sharding: spatial over H -- core k owns rows [64k, 64k+64), host ships a
1-row halo. No cross-core communication.

Device-side design (per core):
  * The 3x3 convs are folded into layer-1: the host stacks x three times on
    partitions with row shifts -1/0/+1 (K = 3*16 = 48) and builds three
    "tap" weight matrices B_dx [128, 48] (dx = -1/0/+1 as free-dim shifts of
    a 514-wide zero-padded row layout), so layer 1 is 3 accumulating
    matmuls per output row and no vector-engine conv work at all.
  * Two row streams run concurrently on the PE via row-tiling (K=48 tiles
    at array rows 0 and 64); layer 2 (K=128, M=16) cycles PSUM output
    partitions 32j so 4 consecutive rows land batch-packed [112, 512] in
    one PSUM bank for a wide epilogue.
  * relu+b1 PSUM->SBUF evacuation is split between ScalarE (stream A) and
    VectorE (stream B); upd = (noise > 0.5) * (w2 h1 + b2) uses a fused
    scalar_tensor_tensor on GpSimd; alive is computed once per core with a
    rows-on-partitions maxpool and broadcast into a persistent SBUF tile.
  * bf16 data path (fp32 PSUM accumulation), bf16 output upcast on host.
"""

import numpy as np
import ml_dtypes

import sys
if "/opt/trn_rl_repo" not in sys.path:
    sys.path.insert(0, "/opt/trn_rl_repo")

import concourse.bass as bass
import concourse.bacc as bacc
import concourse.mybir as mybir
import concourse.tile as tile
from concourse.bass_utils import run_bass_kernel_spmd
from concourse.alu_op_type import AluOpType

BF16 = mybir.dt.bfloat16
F32 = mybir.dt.float32
Alu = mybir.AluOpType if hasattr(mybir, "AluOpType") else AluOpType
ACTF = mybir.ActivationFunctionType

B, C, H, W = 4, 16, 512, 512
HID = 128
NCORES = 8
RS = H // NCORES          # 64 rows per core
RB = 16                   # rows per stream per superblock
WP = W + 2                # padded row stride (zero cols at 0 and 513)
NSB = RS // (2 * RB)      # superblocks per batch (A gets RB rows, B the next RB)
NSEG = RS // 4            # 4-row epilogue groups per batch-column


def _build_bass():
    nc = bacc.Bacc("TRN2", target_bir_lowering=False, debug=False)

    xp = nc.dram_tensor("xp", [B, C, RS + 2, W], BF16, kind="ExternalInput")
    nz = nc.dram_tensor("nz", [B, C, RS, W], F32, kind="ExternalInput")
    w1t = nc.dram_tensor("w1t", [128, 3 * 128], BF16, kind="ExternalInput")
    w2t = nc.dram_tensor("w2t", [128, 16], BF16, kind="ExternalInput")
    b1v = nc.dram_tensor("b1v", [128, 1], F32, kind="ExternalInput")
    b2v = nc.dram_tensor("b2v", [128, 1], F32, kind="ExternalInput")
    alf = nc.dram_tensor("alf", [RS + 2, W], F32, kind="ExternalInput")
    alsc = nc.dram_tensor("alsc", [RS, W], BF16)
    sout = nc.dram_tensor("sout", [B, C, RS, W], BF16, kind="ExternalOutput")

    with tile.TileContext(nc) as tc:
        _emit(nc, tc, xp, nz, w1t, w2t, b1v, b2v, alf, alsc, sout)
    nc.compile()
    return nc


def _emit(nc, tc, xp, nz, w1t, w2t, b1v, b2v, alf, alsc, sout):
    import contextlib
    ctx = contextlib.ExitStack()
    consts = ctx.enter_context(tc.tile_pool(name="consts", bufs=1))
    trips = ctx.enter_context(tc.tile_pool(name="trips", bufs=1))
    h1p = ctx.enter_context(tc.tile_pool(name="h1p", bufs=6))
    epi = ctx.enter_context(tc.tile_pool(name="epi", bufs=3))
    outp = ctx.enter_context(tc.tile_pool(name="outp", bufs=3))
    ppA = ctx.enter_context(tc.tile_pool(name="ppA", bufs=2, space="PSUM"))
    ppB = ctx.enter_context(tc.tile_pool(name="ppB", bufs=2, space="PSUM"))
    pL2A = ctx.enter_context(tc.tile_pool(name="pL2A", bufs=2, space="PSUM"))
    pL2B = ctx.enter_context(tc.tile_pool(name="pL2B", bufs=2, space="PSUM"))

    # ---- constants ----
    w1taps = consts.tile([128, 3 * 128], BF16)
    nc.sync.dma_start(w1taps[:, :], w1t[:, :])
    w2tile = consts.tile([128, 16], BF16)
    nc.sync.dma_start(w2tile[:, :], w2t[:, :])
    b1tile = consts.tile([128, 1], F32)
    nc.sync.dma_start(b1tile[:, :], b1v[:, :])
    b2tile = consts.tile([128, 1], F32)
    nc.sync.dma_start(b2tile[:, :], b2v[:, :])

    # ---- alive mask, computed once per core ----
    # alpha rows (incl. 1-row halo) on partitions, zero-padded columns.
    alpha = consts.tile([RS + 2, WP], F32)
    nc.vector.memset(alpha[:, :], 0.0)
    nc.sync.dma_start(alpha[:, 1:1 + W], alf[:, :])
    mh = consts.tile([RS + 2, W], F32)
    nc.vector.tensor_tensor(mh[:, :], alpha[:, 0:W], alpha[:, 2:2 + W], op=Alu.max)
    nc.vector.tensor_tensor(mh[:, :], mh[:, :], alpha[:, 1:1 + W], op=Alu.max)
    # vertical max3 needs partition shifts -> small SBUF->SBUF DMAs
    mhs1 = consts.tile([RS + 1, W], F32)
    nc.sync.dma_start(mhs1[0:RS + 1, :], mh[1:RS + 2, :])
    mhs2 = consts.tile([RS, W], F32)
    nc.sync.dma_start(mhs2[0:RS, :], mh[2:RS + 2, :])
    mv = consts.tile([RS, W], F32)
    nc.vector.tensor_tensor(mv[:, :], mh[0:RS, :], mhs1[0:RS, :], op=Alu.max)
    nc.vector.tensor_tensor(mv[:, :], mv[:, :], mhs2[0:RS, :], op=Alu.max)
    alive64 = consts.tile([RS, W], BF16)
    nc.vector.tensor_scalar(alive64[:, :], mv[:, :], 0.1, None, op0=Alu.is_gt)
    # round-trip through DRAM to reach the broadcast layout:
    # alivebc[32j+c, s*W + w] = alive(row 4s+j, w), c = 0..15
    nc.sync.dma_start(alsc[:, :], alive64[:, :])
    alivebc = consts.tile([128, NSEG * W], BF16)
    alr = alsc.rearrange("(s j) w -> j s w", j=4)
    for j in range(4):
        src = alr[j].partition_broadcast(16)          # [16, NSEG, W], step-0 bcast
        dst = alivebc[32 * j:32 * j + 16].rearrange("p (s w) -> p s w", w=W)
        nc.sync.dma_start(dst, src)

    # ---- persistent triple-stacked x tiles (ping-pong) ----
    # partition 16j+c (stream A) / 64+16j+c (stream B) holds
    # x[b, c, row_local + j, :] over RB rows, 514-padded free layout.
    trip_tiles = []
    for i in range(2):
        t = trips.tile([128, RB * WP], BF16, name=f"trip{i}")
        nc.vector.memset(t[:, :], 0.0)
        trip_tiles.append(t)

    def load_trip(t, b, q):
        # stream A rows q..q+RB-1, stream B rows q+RB..q+2RB-1 (core-local)
        for half, base, q0 in ((0, 0, q), (1, 64, q + RB)):
            for j in range(3):
                dst = t[base + 16 * j: base + 16 * j + 16].rearrange(
                    "p (r w) -> p r w", w=WP
                )[:, :, 1:1 + W]
                nc.sync.dma_start(dst, xp[b, :, q0 + j: q0 + j + RB, :])

    # ---- steady-state pipeline ----
    sbi = 0
    for b in range(B):
        for sb in range(NSB):
            q = sb * 2 * RB
            t = trip_tiles[sbi % 2]
            sbi += 1
            load_trip(t, b, q)
            l2A = l2B = None
            for rr in range(RB):
                jj = rr % 4
                psA = ppA.tile([128, 512], F32, tag="psA")
                psB = ppB.tile([128, 512], F32, tag="psB")
                # layer 1: 3 taps, streams A/B interleaved for row-tile overlap
                for tap in range(3):
                    nc.tensor.matmul(
                        psA[:, :],
                        w1taps[0:48, 128 * tap:128 * tap + 128],
                        t[0:48, rr * WP + tap: rr * WP + tap + 512],
                        start=(tap == 0), stop=(tap == 2),
                    )
                    nc.tensor.matmul(
                        psB[:, :],
                        w1taps[64:112, 128 * tap:128 * tap + 128],
                        t[64:112, rr * WP + tap: rr * WP + tap + 512],
                        start=(tap == 0), stop=(tap == 2),
                    )
                # relu + b1 evacuation, split across ScalarE / VectorE
                h1A = h1p.tile([128, 512], BF16, tag="h1A")
                nc.scalar.activation(h1A[:, :], psA[:, :], ACTF.Relu, bias=b1tile[:, 0:1])
                h1B = h1p.tile([128, 512], BF16, tag="h1B")
                nc.vector.tensor_scalar(
                    h1B[:, :], psB[:, :], b1tile[:, 0:1], 0.0, op0=Alu.add, op1=Alu.max
                )
                # layer 2 into batch-packed psum (partitions 32j..32j+15)
                if jj == 0:
                    l2A = pL2A.tile([128, 512], F32, tag="l2A")
                    l2B = pL2B.tile([128, 512], F32, tag="l2B")
                nc.tensor.matmul(l2A[32 * jj:32 * jj + 16, :], w2tile[:, :], h1A[:, :],
                                 tile_position=(0, 32 * jj))
                nc.tensor.matmul(l2B[32 * jj:32 * jj + 16, :], w2tile[:, :], h1B[:, :],
                                 tile_position=(0, 32 * jj))
                if jj == 3:
                    for stream, l2ps in (("A", l2A), ("B", l2B)):
                        rowg = q + (rr - 3) if stream == "A" else q + RB + (rr - 3)
                        seg = rowg // 4
                        u = epi.tile([112, 512], BF16, tag="u")
                        nc.scalar.activation(
                            u[:, :], l2ps[0:112, :], ACTF.Identity, bias=b2tile[0:112, 0:1]
                        )
                        nzt = epi.tile([112, 512], F32, tag="nzt")
                        xe = epi.tile([112, 512], BF16, tag="xe")
                        for j in range(4):
                            nc.sync.dma_start(nzt[32 * j:32 * j + 16, :], nz[b, :, rowg + j, :])
                            nc.sync.dma_start(xe[32 * j:32 * j + 16, :], xp[b, :, rowg + j + 1, :])
                        s1 = epi.tile([112, 512], BF16, tag="s1")
                        nc.vector.scalar_tensor_tensor(
                            s1[:, :], nzt[:, :], 0.5, u[:, :], op0=Alu.is_gt, op1=Alu.mult
                        )
                        s2 = epi.tile([112, 512], BF16, tag="s2")
                        nc.gpsimd.tensor_tensor(s2[:, :], s1[:, :], xe[:, :], op=Alu.add)
                        s3 = outp.tile([112, 512], BF16, tag="s3")
                        nc.vector.tensor_tensor(
                            s3[:, :], s2[:, :], alivebc[0:112, seg * W:(seg + 1) * W],
                            op=Alu.mult,
                        )
                        for j in range(4):
                            nc.sync.dma_start(sout[b, :, rowg + j, :], s3[32 * j:32 * j + 16, :])
    ctx.close()


_NC_CACHE = {}


def _get_nc():
    if "nc" not in _NC_CACHE:
        _NC_CACHE["nc"] = _build_bass()
    return _NC_CACHE["nc"]


def _host_weights(w1, b1, w2, b2):
    w1 = np.asarray(w1, np.float32)
    w2 = np.asarray(w2, np.float32)
    b1 = np.asarray(b1, np.float32)
    b2 = np.asarray(b2, np.float32)
    sx = np.outer([1.0, 2.0, 1.0], [-1.0, 0.0, 1.0]) / 8.0   # [dy, dx]
    sy = sx.T
    w1x, w1y, w1i = w1[:, :16], w1[:, 16:32], w1[:, 32:48]
    # B_dx[:, 16j+c] = w1x*sx[j,dx] + w1y*sy[j,dx] + w1i*(j==1 and dx==1)
    w1taps = np.zeros((128, 3 * 128), np.float32)
    for dx in range(3):
        Bdx = np.zeros((128, 48), np.float32)
        for j in range(3):
            A = w1x * sx[j, dx] + w1y * sy[j, dx]
            if j == 1 and dx == 1:
                A = A + w1i
            Bdx[:, 16 * j:16 * j + 16] = A
        # lhsT = Bdx.T at partition rows 0-47 (stream A) and 64-111 (stream B)
        w1taps[0:48, 128 * dx:128 * dx + 128] = Bdx.T
        w1taps[64:112, 128 * dx:128 * dx + 128] = Bdx.T
    w2t = w2.T.copy()                                        # [128, 16]
    b1v = b1.reshape(128, 1)
    b2v = np.zeros((128, 1), np.float32)
    for j in range(4):
        b2v[32 * j:32 * j + 16, 0] = b2
    return (
        w1taps.astype(ml_dtypes.bfloat16),
        w2t.astype(ml_dtypes.bfloat16),
        b1v, b2v,
    )


def kernel(x, noise, w1, b1, w2, b2):
    x = np.asarray(x, np.float32)
    noise = np.asarray(noise, np.float32)
    nc = _get_nc()
    w1taps, w2t, b1v, b2v = _host_weights(w1, b1, w2, b2)

    xbf = x.astype(ml_dtypes.bfloat16)
    xpad = np.zeros((B, C, H + 2, W), ml_dtypes.bfloat16)
    xpad[:, :, 1:H + 1, :] = xbf
    # alive threshold compares alpha in fp32 for exactness
    apad = np.zeros((H + 2, W), np.float32)
    apad[1:H + 1, :] = x[0, 3]

    in_maps = []
    for k in range(NCORES):
        r0 = k * RS
        in_maps.append({
            "xp": np.ascontiguousarray(xpad[:, :, r0:r0 + RS + 2, :]),
            "nz": np.ascontiguousarray(noise[:, :, r0:r0 + RS, :]),
            "w1t": w1taps, "w2t": w2t, "b1v": b1v, "b2v": b2v,
            "alf": np.ascontiguousarray(apad[r0:r0 + RS + 2, :]),
        })

    res = run_bass_kernel_spmd(nc, in_maps, list(range(NCORES)))
    state = np.empty((B, C, H, W), np.float32)
    for k in range(NCORES):
        state[:, :, k * RS:(k + 1) * RS, :] = np.asarray(
            res.results[k]["sout"], dtype=np.float32
        )
    visible = state[:, :4]
    return state, visible
